# revision 40
# baseline (speedup 1.0000x reference)
"""Trainium2 Bass kernel for the note/wiki 3-way contraction + gate MLP.

Math (per note n):
    e[n]    = (wikivec * notevec[n]) @ W_emb.T + b_emb          # (C, K)
    attn[n] = sigmoid(e[n] @ W_att.T + b_att)                   # (C, K)
    s[n]    = sum_k attn[n]*e[n]*W_out[0,k] + b_out             # (C,)

Sharding: data-parallel over the 16 notes -> 2 notes per core on 8 cores
(wikivec / W_emb replicated).  All device data rides ONE v-tile-major bf16
stream laid out host-side as the exact SBUF image: per v-tile j, 512
columns = [wikiT(256) | wembT(256)].  The f32 note scales are bitcast into
the first block and the phase-2 constants (W_att^T, W_out, biases) into
the last block, so there are no extra DMAs / semaphore lanes.

Phase 1 per v-tile: mov[128, l*256+c] = wikiT[v,c] * note_l[v] via two
[128,256] tensor_scalar ops, BOTH on DVE (ACT's ~350ns fixed op overhead
made it the production bottleneck; DVE runs these at ~197ns in 4x mode,
so 2 ops = 394ns < the 432ns the PE needs per v-tile).  Then 2 matmuls
(k-halves) accumulate e^T[k, (l,c)] into two PSUM banks over 79 v-tiles.

Dummy matmuls on a memset tile bridge the ~4us DMA pipe latency of the
first block AND hold the PE clock-gate (HAM) at full rate; a dummy
sigmoid preloads the ACT function-table set (sigmoid_and_others, which
also carries Identity/Copy) so phase 2 never stalls on a table load.

Phase 2: eb = bf16(e + b_emb) split DVE / ACT-Identity (parallel), 4
matmuls for attn logits (jm-outer so sigmoid 0 starts after two), sigmoid
(+b_att) on ACT, gate on DVE (bf16), W_out contraction on PE, PSUM->SBUF
copy on ACT, DMA out s [1, 512].  b_out is added host-side in gather().
"""

import sys

if "/opt/trn_rl_repo" not in sys.path:
    sys.path.insert(0, "/opt/trn_rl_repo")

import numpy as np
import ml_dtypes

import concourse.bass as bass
import concourse.mybir as mybir
import concourse.tile as tile
from concourse import bacc
from concourse.bass_utils import run_bass_kernel_spmd

N_CORES = 8
N, C, V, K = 16, 256, 10000, 256
J = 79  # number of 128-row v-tiles (V zero-padded to 10112)
NLOC = 2  # notes per core
CLOC = C  # codes per core (replicated)
NC2 = NLOC * C  # 512 moving columns (col = l*256 + c)
COLS = C + K  # 512 stream columns per v-tile: [wikiT(256) | wembT(256)]
SCW = NLOC * J  # 158 f32 scale words per partition
WATW = 2 * K + 2  # attn stationary + W_out columns (bf16)
CFW = 6  # f32 const words: bemb(2) batt(2) bout(1) pad(1)
# DMA blocks: (extra_segment, n_v_tiles).  All SBUF-resident, all on the
# single Sync HWDGE ring so delivery follows consumption order (the two
# rings share one HBM path; interleaving delivers tiles out of order).
# Small head blocks so the first semaphores land just after the ~3us DMA
# pipe latency; unique per-block pool tags keep every DMA "ready" at
# schedule time, which preserves program-order issue (a rotating pool
# throttles mid blocks and lets the scheduler hoist late ones).
BLOCKS = [("sc", 2), (None, 2), (None, 4)] + [(None, 8)] * 8 + [("cst", 7)]
assert sum(nb for _, nb in BLOCKS) == J
N_WARM_MM = 16  # dummy PE matmuls bridging the first-block DMA latency

F32 = mybir.dt.float32
BF16 = mybir.dt.bfloat16
BF16_NP = ml_dtypes.bfloat16

_NC_CACHE = {}


def _build_nc():
    nc = bacc.Bacc(None, target_bir_lowering=False)

    # stream columns: [sc bitcast (2*SCW) | blocks of v-tiles | watx | cf]
    TOTC = 2 * SCW + J * COLS + WATW + 2 * CFW
    stream = nc.declare_dram_parameter("stream", [128, TOTC], BF16, isOutput=False)
    s_out = nc.declare_dram_parameter("s_out", [1, NC2], F32, isOutput=True)

    SIG = mybir.ActivationFunctionType.Sigmoid

    with tile.TileContext(nc) as tc:
        with (
            tc.tile_pool(name="const", bufs=1) as constp,
            tc.tile_pool(name="st", bufs=len(BLOCKS)) as stp,
            tc.tile_pool(name="mov", bufs=8) as movp,
            tc.tile_pool(name="post", bufs=1) as postp,
            tc.tile_pool(name="psum", bufs=1, space="PSUM") as psp,
        ):
            # ---- warmups (no DMA deps): PE clock gate + ACT sigmoid table
            warm = constp.tile([128, 256], BF16)
            nc.vector.memset(warm[:], 0.5)
            warm_ps = psp.tile([128, 256], F32, tag="warm_ps")
            for _ in range(N_WARM_MM):
                nc.tensor.matmul(
                    warm_ps[:], warm[:, 0:128], warm[:], start=True, stop=True
                )
            # preload the sigmoid table set (also covers Identity/Copy)
            warm_sig = constp.tile([128, 1], F32)
            nc.scalar.activation(warm_sig[:], warm[:, 0:1], SIG, bias=0.0, scale=1.0)

            # e^T accumulators: [k-half 128, (l,c) 512] fp32, one bank each
            e_ps = [
                psp.tile([128, NC2], F32, name=f"e_ps{m}", tag=f"e_ps{m}")
                for m in range(2)
            ]

            sc = None  # f32 view of the note scales
            wat = None  # phase-2 constants
            cfs = None
            j = 0
            off = 0
            for b, (seg, nb) in enumerate(BLOCKS):
                pre = {None: 0, "sc": 2 * SCW, "cst": WATW + 2 * CFW}[seg]
                w = pre + nb * COLS
                st = stp.tile([128, w], BF16, tag=f"st{b}", bufs=1)
                # one ring (Sync) for the whole stream: the two HWDGE rings
                # share the same HBM path, and packet round-robin across
                # rings would deliver tiles OUT of consumption order
                nc.sync.dma_start(st[:], stream[:, off : off + w])
                off += w
                if seg == "sc":
                    sc = st[:, 0 : 2 * SCW].bitcast(F32)
                elif seg == "cst":
                    wat = st[:, 0:WATW]
                    cfs = st[:, WATW : WATW + 2 * CFW].bitcast(F32)
                for jj in range(nb):
                    base = pre + jj * COLS
                    wk = st[:, base : base + C]
                    mov = movp.tile([128, NC2], BF16)
                    for l in range(NLOC):
                        nc.vector.tensor_scalar_mul(
                            mov[:, l * C : (l + 1) * C],
                            wk,
                            sc[:, l * J + j : l * J + j + 1],
                        )
                    st_, sp_ = (j == 0), (j == J - 1)
                    for m in range(2):
                        nc.tensor.matmul(
                            e_ps[m][:],
                            st[:, base + C + m * 128 : base + C + (m + 1) * 128],
                            mov[:],
                            start=st_,
                            stop=sp_,
                        )
                    j += 1

            # ---- phase 2 ----
            # eb0 on DVE, eb1 on ACT (Identity lives in the same preloaded
            # table set as Sigmoid) so the two bias-casts run in parallel
            IDN = mybir.ActivationFunctionType.Identity
            eb0 = postp.tile([128, NC2], BF16, tag="eb0")
            nc.vector.tensor_scalar_add(eb0[:], e_ps[0][:], cfs[:, 0:1])
            eb1 = postp.tile([128, NC2], BF16, tag="eb1")
            nc.scalar.activation(eb1[:], e_ps[1][:], IDN, bias=cfs[:, 1:2], scale=1.0)
            eb = [eb0, eb1]

            a_ps = [
                psp.tile([128, NC2], F32, name=f"a_ps{jm}", tag=f"a_ps{jm}")
                for jm in range(2)
            ]
            # jm-outer so a_ps[0] completes after two matmuls and sigmoid 0
            # starts as early as possible
            for jm in range(2):
                for kt in range(2):
                    nc.tensor.matmul(
                        a_ps[jm][:],
                        wat[:, kt * K + jm * 128 : kt * K + (jm + 1) * 128],
                        eb[kt][:],
                        start=(kt == 0),
                        stop=(kt == 1),
                    )

            v = []
            for jm in range(2):
                atn = postp.tile([128, NC2], BF16, tag=f"atn{jm}")
                nc.scalar.activation(
                    atn[:], a_ps[jm][:], SIG, bias=cfs[:, 2 + jm : 3 + jm], scale=1.0
                )
                v_jm = postp.tile([128, NC2], BF16, tag=f"v{jm}")
                nc.vector.tensor_mul(v_jm[:], atn[:], eb[jm][:])
                v.append(v_jm)

            s_ps = psp.tile([1, NC2], F32, tag="s_ps")
            for kt in range(2):
                nc.tensor.matmul(
                    s_ps[:],
                    wat[:, 2 * K + kt : 2 * K + kt + 1],
                    v[kt][:],
                    start=(kt == 0),
                    stop=(kt == 1),
                )
            # b_out is added host-side; ACT Copy is the cheapest [1,512]
            # PSUM->SBUF move and ACT is idle here (DVE still gating)
            # PSUM->SBUF move on ACT (idle after the sigmoids)
            s_sb = postp.tile([1, NC2], F32, tag="s_sb")
            nc.scalar.copy(s_sb[:], s_ps[:])
            # out rides the ACT ring: the Sync ring's FIFO served the whole
            # input stream and its issue slot can lag
            nc.scalar.dma_start(s_out[:], s_sb[:], single_packet=True)

    nc.compile()
    return nc


def _get_nc():
    if "nc" not in _NC_CACHE:
        _NC_CACHE["nc"] = _build_nc()
    return _NC_CACHE["nc"]


def prep_inputs(notevec, wikivec, W_emb, b_emb, W_att, b_att, W_out, b_out):
    notevec = np.asarray(notevec, np.float32)
    wikivec = np.asarray(wikivec, np.float32)
    W_emb = np.asarray(W_emb, np.float32)

    # v-tile images, v-major [J*128, COLS]: [wikiT | wembT] (zero-padded v)
    img = np.zeros((J * 128, COLS), BF16_NP)
    img[:V, 0:C] = wikivec.T.astype(BF16_NP)
    img[:V, C:] = W_emb.T.astype(BF16_NP)
    tiles = np.ascontiguousarray(
        img.reshape(J, 128, COLS).transpose(1, 0, 2)
    ).reshape(128, J * COLS)

    # scales[p, l*J + j] = notevec[2i+l, j*128 + p], f32 viewed as bf16 pairs
    nv = np.zeros((N, J * 128), np.float32)
    nv[:, :V] = notevec

    # attn stationary [kp, kt*256 + jcol] plus W_out columns [kp, kt]
    watk = np.ascontiguousarray(
        np.asarray(W_att, np.float32).T.reshape(2, 128, K).transpose(1, 0, 2)
    ).reshape(128, 2 * K)
    wo = np.asarray(W_out, np.float32)[0].reshape(2, 128).T
    watx = np.concatenate([watk, wo], axis=1).astype(BF16_NP)

    cfh = np.zeros((128, CFW), np.float32)
    cfh[:, 0:2] = np.asarray(b_emb, np.float32).reshape(2, 128).T
    cfh[:, 2:4] = np.asarray(b_att, np.float32).reshape(2, 128).T
    cfh[:, 4] = np.asarray(b_out, np.float32)[0]
    cf_bf = cfh.view(BF16_NP)  # [128, 2*CFW] raw bytes

    in_maps = []
    for i in range(N_CORES):
        sch = np.ascontiguousarray(
            nv[i * NLOC : (i + 1) * NLOC].reshape(NLOC, J, 128).transpose(2, 0, 1)
        ).reshape(128, SCW)
        sc_bf = sch.view(BF16_NP)  # [128, 2*SCW] raw bytes
        segs = []
        j = 0
        for seg, nb in BLOCKS:
            if seg == "sc":
                segs.append(sc_bf)
            elif seg == "cst":
                segs.append(watx)
                segs.append(cf_bf)
            segs.append(tiles[:, j * COLS : (j + nb) * COLS])
            j += nb
        strm = np.concatenate(segs, axis=1)
        in_maps.append({"stream": np.ascontiguousarray(strm)})
    return in_maps


def run(in_maps, **kw):
    nc = _get_nc()
    return run_bass_kernel_spmd(nc, in_maps, list(range(N_CORES)), **kw)


def gather(results, bout=0.0):
    out = np.zeros((N, C), np.float32)
    for i, r in enumerate(results):
        out[i * NLOC : (i + 1) * NLOC, :] = r["s_out"].reshape(NLOC, C)
    return out + np.float32(bout)


def kernel(notevec, wikivec, W_emb, b_emb, W_att, b_att, W_out, b_out):
    in_maps = prep_inputs(
        notevec, wikivec, W_emb, b_emb, W_att, b_att, W_out, b_out
    )
    res = run(in_maps)
    return gather(res.results, float(np.asarray(b_out).reshape(-1)[0]))


# revision 41
# speedup vs baseline: 1.0277x; 1.0277x over previous
"""Trainium2 Bass kernel for the note/wiki 3-way contraction + gate MLP.

Math (per note n):
    e[n]    = (wikivec * notevec[n]) @ W_emb.T + b_emb          # (C, K)
    attn[n] = sigmoid(e[n] @ W_att.T + b_att)                   # (C, K)
    s[n]    = sum_k attn[n]*e[n]*W_out[0,k] + b_out             # (C,)

Sharding: data-parallel over the 16 notes -> 2 notes per core on 8 cores
(wikivec / W_emb replicated).  All device data rides ONE v-tile-major bf16
stream laid out host-side as the exact SBUF image: per v-tile j, 512
columns = [wikiT(256) | wembT(256)].  The f32 note scales are bitcast into
the first block and the phase-2 constants (W_att^T, W_out, biases) into
the last block, so there are no extra DMAs / semaphore lanes.

Phase 1 per v-tile: mov[128, l*256+c] = wikiT[v,c] * note_l[v] via two
[128,256] tensor_scalar ops, BOTH on DVE (ACT's ~350ns fixed op overhead
made it the production bottleneck; DVE runs these at ~197ns in 4x mode,
so 2 ops = 394ns < the 432ns the PE needs per v-tile).  Then 2 matmuls
(k-halves) accumulate e^T[k, (l,c)] into two PSUM banks over 79 v-tiles.

Dummy matmuls on a memset tile bridge the ~4us DMA pipe latency of the
first block AND hold the PE clock-gate (HAM) at full rate; a dummy
sigmoid preloads the ACT function-table set (sigmoid_and_others, which
also carries Identity/Copy) so phase 2 never stalls on a table load.

Phase 2: eb = bf16(e + b_emb) split DVE / ACT-Identity (parallel), 4
matmuls for attn logits (jm-outer so sigmoid 0 starts after two), sigmoid
(+b_att) on ACT, gate on DVE (bf16), W_out contraction on PE, PSUM->SBUF
copy on ACT, DMA out s [1, 512].  b_out is added host-side in gather().
"""

import sys

if "/opt/trn_rl_repo" not in sys.path:
    sys.path.insert(0, "/opt/trn_rl_repo")

import numpy as np
import ml_dtypes

import concourse.bass as bass
import concourse.mybir as mybir
import concourse.tile as tile
from concourse import bacc
from concourse.bass_utils import run_bass_kernel_spmd

N_CORES = 8
N, C, V, K = 16, 256, 10000, 256
J = 79  # number of 128-row v-tiles (V zero-padded to 10112)
NLOC = 2  # notes per core
CLOC = C  # codes per core (replicated)
NC2 = NLOC * C  # 512 moving columns (col = l*256 + c)
COLS = C + K  # 512 stream columns per v-tile: [wikiT(256) | wembT(256)]
SCW = NLOC * J  # 158 f32 scale words per partition
WATW = 2 * K + 2  # attn stationary + W_out columns (bf16)
CFW = 6  # f32 const words: bemb(2) batt(2) bout(1) pad(1)
# DMA blocks: (extra_segment, n_v_tiles).  All SBUF-resident, all on the
# single Sync HWDGE ring so delivery follows consumption order (the two
# rings share one HBM path; interleaving delivers tiles out of order).
# Small head blocks so the first semaphores land just after the ~3us DMA
# pipe latency; unique per-block pool tags keep every DMA "ready" at
# schedule time, which preserves program-order issue (a rotating pool
# throttles mid blocks and lets the scheduler hoist late ones).
BLOCKS = [("sc", 2), (None, 2), (None, 4)] + [(None, 8)] * 8 + [("cst", 7)]
assert sum(nb for _, nb in BLOCKS) == J
N_WARM_MM = 18  # dummy PE matmuls bridging the first-block DMA latency

F32 = mybir.dt.float32
BF16 = mybir.dt.bfloat16
BF16_NP = ml_dtypes.bfloat16

_NC_CACHE = {}


def _build_nc():
    nc = bacc.Bacc(None, target_bir_lowering=False)

    # stream columns: [sc bitcast (2*SCW) | blocks of v-tiles | watx | cf]
    TOTC = 2 * SCW + J * COLS + WATW + 2 * CFW
    stream = nc.declare_dram_parameter("stream", [128, TOTC], BF16, isOutput=False)
    s_out = nc.declare_dram_parameter("s_out", [1, NC2], F32, isOutput=True)

    SIG = mybir.ActivationFunctionType.Sigmoid

    with tile.TileContext(nc) as tc:
        with (
            tc.tile_pool(name="const", bufs=1) as constp,
            tc.tile_pool(name="st", bufs=len(BLOCKS)) as stp,
            tc.tile_pool(name="mov", bufs=8) as movp,
            tc.tile_pool(name="post", bufs=1) as postp,
            tc.tile_pool(name="psum", bufs=1, space="PSUM") as psp,
        ):
            # ---- warmups (no DMA deps): PE clock gate + ACT sigmoid table
            warm = constp.tile([128, 256], BF16)
            nc.vector.memset(warm[:], 0.5)
            warm_ps = psp.tile([128, 256], F32, tag="warm_ps")
            for _ in range(N_WARM_MM):
                nc.tensor.matmul(
                    warm_ps[:], warm[:, 0:128], warm[:], start=True, stop=True
                )
            # preload the sigmoid table set (also covers Identity/Copy)
            warm_sig = constp.tile([128, 1], F32)
            nc.scalar.activation(warm_sig[:], warm[:, 0:1], SIG, bias=0.0, scale=1.0)

            # e^T accumulators: [k-half 128, (l,c) 512] fp32, one bank each
            e_ps = [
                psp.tile([128, NC2], F32, name=f"e_ps{m}", tag=f"e_ps{m}")
                for m in range(2)
            ]

            sc = None  # f32 view of the note scales
            wat = None  # phase-2 constants
            cfs = None
            j = 0
            off = 0
            for b, (seg, nb) in enumerate(BLOCKS):
                pre = {None: 0, "sc": 2 * SCW, "cst": WATW + 2 * CFW}[seg]
                w = pre + nb * COLS
                st = stp.tile([128, w], BF16, tag=f"st{b}", bufs=1)
                # one ring (Sync) for the whole stream: the two HWDGE rings
                # share the same HBM path, and packet round-robin across
                # rings would deliver tiles OUT of consumption order
                nc.sync.dma_start(st[:], stream[:, off : off + w])
                off += w
                if seg == "sc":
                    sc = st[:, 0 : 2 * SCW].bitcast(F32)
                elif seg == "cst":
                    wat = st[:, 0:WATW]
                    cfs = st[:, WATW : WATW + 2 * CFW].bitcast(F32)
                for jj in range(nb):
                    base = pre + jj * COLS
                    wk = st[:, base : base + C]
                    mov = movp.tile([128, NC2], BF16)
                    for l in range(NLOC):
                        nc.vector.tensor_scalar_mul(
                            mov[:, l * C : (l + 1) * C],
                            wk,
                            sc[:, l * J + j : l * J + j + 1],
                        )
                    st_, sp_ = (j == 0), (j == J - 1)
                    for m in range(2):
                        nc.tensor.matmul(
                            e_ps[m][:],
                            st[:, base + C + m * 128 : base + C + (m + 1) * 128],
                            mov[:],
                            start=st_,
                            stop=sp_,
                        )
                    j += 1

            # ---- phase 2 ----
            # eb0 on DVE, eb1 on ACT (Identity lives in the same preloaded
            # table set as Sigmoid) so the two bias-casts run in parallel
            IDN = mybir.ActivationFunctionType.Identity
            eb0 = postp.tile([128, NC2], BF16, tag="eb0")
            nc.vector.tensor_scalar_add(eb0[:], e_ps[0][:], cfs[:, 0:1])
            eb1 = postp.tile([128, NC2], BF16, tag="eb1")
            nc.scalar.activation(eb1[:], e_ps[1][:], IDN, bias=cfs[:, 1:2], scale=1.0)
            eb = [eb0, eb1]

            a_ps = [
                psp.tile([128, NC2], F32, name=f"a_ps{jm}", tag=f"a_ps{jm}")
                for jm in range(2)
            ]
            # jm-outer so a_ps[0] completes after two matmuls and sigmoid 0
            # starts as early as possible
            for jm in range(2):
                for kt in range(2):
                    nc.tensor.matmul(
                        a_ps[jm][:],
                        wat[:, kt * K + jm * 128 : kt * K + (jm + 1) * 128],
                        eb[kt][:],
                        start=(kt == 0),
                        stop=(kt == 1),
                    )

            v = []
            for jm in range(2):
                atn = postp.tile([128, NC2], BF16, tag=f"atn{jm}")
                nc.scalar.activation(
                    atn[:], a_ps[jm][:], SIG, bias=cfs[:, 2 + jm : 3 + jm], scale=1.0
                )
                v_jm = postp.tile([128, NC2], BF16, tag=f"v{jm}")
                nc.vector.tensor_mul(v_jm[:], atn[:], eb[jm][:])
                v.append(v_jm)

            s_ps = psp.tile([1, NC2], F32, tag="s_ps")
            for kt in range(2):
                nc.tensor.matmul(
                    s_ps[:],
                    wat[:, 2 * K + kt : 2 * K + kt + 1],
                    v[kt][:],
                    start=(kt == 0),
                    stop=(kt == 1),
                )
            # b_out is added host-side; ACT Copy is the cheapest [1,512]
            # PSUM->SBUF move and ACT is idle here (DVE still gating)
            # PSUM->SBUF move on ACT (idle after the sigmoids)
            s_sb = postp.tile([1, NC2], F32, tag="s_sb")
            nc.scalar.copy(s_sb[:], s_ps[:])
            # out rides the ACT ring: the Sync ring's FIFO served the whole
            # input stream and its issue slot can lag
            nc.scalar.dma_start(s_out[:], s_sb[:], single_packet=True)

    nc.compile()
    return nc


def _get_nc():
    if "nc" not in _NC_CACHE:
        _NC_CACHE["nc"] = _build_nc()
    return _NC_CACHE["nc"]


def prep_inputs(notevec, wikivec, W_emb, b_emb, W_att, b_att, W_out, b_out):
    notevec = np.asarray(notevec, np.float32)
    wikivec = np.asarray(wikivec, np.float32)
    W_emb = np.asarray(W_emb, np.float32)

    # v-tile images, v-major [J*128, COLS]: [wikiT | wembT] (zero-padded v)
    img = np.zeros((J * 128, COLS), BF16_NP)
    img[:V, 0:C] = wikivec.T.astype(BF16_NP)
    img[:V, C:] = W_emb.T.astype(BF16_NP)
    tiles = np.ascontiguousarray(
        img.reshape(J, 128, COLS).transpose(1, 0, 2)
    ).reshape(128, J * COLS)

    # scales[p, l*J + j] = notevec[2i+l, j*128 + p], f32 viewed as bf16 pairs
    nv = np.zeros((N, J * 128), np.float32)
    nv[:, :V] = notevec

    # attn stationary [kp, kt*256 + jcol] plus W_out columns [kp, kt]
    watk = np.ascontiguousarray(
        np.asarray(W_att, np.float32).T.reshape(2, 128, K).transpose(1, 0, 2)
    ).reshape(128, 2 * K)
    wo = np.asarray(W_out, np.float32)[0].reshape(2, 128).T
    watx = np.concatenate([watk, wo], axis=1).astype(BF16_NP)

    cfh = np.zeros((128, CFW), np.float32)
    cfh[:, 0:2] = np.asarray(b_emb, np.float32).reshape(2, 128).T
    cfh[:, 2:4] = np.asarray(b_att, np.float32).reshape(2, 128).T
    cfh[:, 4] = np.asarray(b_out, np.float32)[0]
    cf_bf = cfh.view(BF16_NP)  # [128, 2*CFW] raw bytes

    in_maps = []
    for i in range(N_CORES):
        sch = np.ascontiguousarray(
            nv[i * NLOC : (i + 1) * NLOC].reshape(NLOC, J, 128).transpose(2, 0, 1)
        ).reshape(128, SCW)
        sc_bf = sch.view(BF16_NP)  # [128, 2*SCW] raw bytes
        segs = []
        j = 0
        for seg, nb in BLOCKS:
            if seg == "sc":
                segs.append(sc_bf)
            elif seg == "cst":
                segs.append(watx)
                segs.append(cf_bf)
            segs.append(tiles[:, j * COLS : (j + nb) * COLS])
            j += nb
        strm = np.concatenate(segs, axis=1)
        in_maps.append({"stream": np.ascontiguousarray(strm)})
    return in_maps


def run(in_maps, **kw):
    nc = _get_nc()
    return run_bass_kernel_spmd(nc, in_maps, list(range(N_CORES)), **kw)


def gather(results, bout=0.0):
    out = np.zeros((N, C), np.float32)
    for i, r in enumerate(results):
        out[i * NLOC : (i + 1) * NLOC, :] = r["s_out"].reshape(NLOC, C)
    return out + np.float32(bout)


def kernel(notevec, wikivec, W_emb, b_emb, W_att, b_att, W_out, b_out):
    in_maps = prep_inputs(
        notevec, wikivec, W_emb, b_emb, W_att, b_att, W_out, b_out
    )
    res = run(in_maps)
    return gather(res.results, float(np.asarray(b_out).reshape(-1)[0]))
